# revision 13
# baseline (speedup 1.0000x reference)
"""Trainium2 Bass kernel for nn_Attention_60224031424957.

Attention block: x[8,384,32,32] -> 1x1 conv QKV -> 12-head attention with
relative-position bias (rpb_table[rel_index]) -> softmax -> proj.

Sharding: i-block data parallel. Core c owns query columns i in
[128c, 128c+128) for ALL batches/heads; k/v are computed per-core for all
batches (duplicated). The bias gather (the dominant cost: 131072 random
table rows per core) runs on GPSIMD via ap_gather with one head per
channel, then a DRAM bounce rearranges it to j-partitioned planes.

Layouts on device (per core):
  S^T[j, i] tiles: psum [128 j-in-tile, (jt,i)] per head; softmax over j via
  a ones-column appended to V (row 32 of the O^T psum = denominators).
  P^T = exp(scale*S^T) (*) exp(biasT)   (bf16)
  O^T = V_aug^T @ P^T accumulated over j-tiles; normalized by 1/sums.
"""

import numpy as np
import ml_dtypes

import concourse.bacc as bacc
import concourse.mybir as mybir
import concourse.tile as tile
from concourse.tile import add_dep_helper
from concourse import bass_utils
from concourse._compat import get_trn_type

BF16 = ml_dtypes.bfloat16

B, C, Hh, Ww = 8, 384, 32, 32
N = Hh * Ww            # 1024
HEADS = 12
HD = 32                # head dim
NTAB = (2 * Hh - 1) * (2 * Ww - 1)  # 3969
NCORES = 8
NI = N // NCORES       # 128 query columns per core
NJT = 8                # j tiles of 128
SCALE = HD ** -0.5

F32 = mybir.dt.float32
BF = mybir.dt.bfloat16
I16 = mybir.dt.int16

_CACHE = {}


def build_nc(nb=B):
    nc = bacc.Bacc(get_trn_type() or "TRN2", target_bir_lowering=False, debug=False)

    # ---- DRAM inputs ----
    xf = nc.dram_tensor("xf", [nb, 3, 128, N], BF, kind="ExternalInput")      # full x, [b, kt, c_in_tile, n]
    xq = nc.dram_tensor("xq", [nb, 3, 128, NI], BF, kind="ExternalInput")     # per-core x slice for q
    qwT = nc.dram_tensor("qwT", [3, 128, C], BF, kind="ExternalInput")        # q_w.T tiles [kt, c, dout]
    kwT = nc.dram_tensor("kwT", [3, 128, C], BF, kind="ExternalInput")
    vwT = nc.dram_tensor("vwT", [3, 128, C], BF, kind="ExternalInput")
    pw4 = nc.dram_tensor("pw4", [32, HEADS, 3, 128], BF, kind="ExternalInput")  # proj_w [d, h, mt, dout]
    qb_r = nc.dram_tensor("qb_r", [128, 3], F32, kind="ExternalInput")
    kb_r = nc.dram_tensor("kb_r", [128, 3], F32, kind="ExternalInput")
    pb_r = nc.dram_tensor("pb_r", [128, 3], F32, kind="ExternalInput")
    vb = nc.dram_tensor("vb", [1, C], BF, kind="ExternalInput")               # kv_b[384:]
    tabT = nc.dram_tensor("tabT", [128, NTAB], F32, kind="ExternalInput")     # table col h at chan 16g+h
    idx16 = nc.dram_tensor("idx16", [128, 1024], I16, kind="ExternalInput")
    # idx16: [128, 1024] int16, group g rows 16g..16g+15, wrapped per-16

    out = nc.dram_tensor("out", [nb, 3, 128, NI], F32, kind="ExternalOutput")
    bounce = nc.dram_tensor("bounce", [NJT, 16, 128, NI], F32)  # [g, chan16, jp, i]

    NCELL = NJT * 128 * NI      # 131072 cells per core
    NPG = NCELL // NJT          # 16384 per group

    with tile.TileContext(nc) as tc:
        with tc.tile_pool(name="const", bufs=1) as cpool, \
             tc.tile_pool(name="gbuf", bufs=1) as gpool, \
             tc.tile_pool(name="work", bufs=2) as wpool, \
             tc.tile_pool(name="attn", bufs=3) as apool, \
             tc.tile_pool(name="pbig", bufs=2, space="PSUM") as pbig, \
             tc.tile_pool(name="potp", bufs=2, space="PSUM") as potp, \
             tc.tile_pool(name="pbcp", bufs=1, space="PSUM") as pbcp, \
             tc.tile_pool(name="psmall", bufs=1, space="PSUM") as psmall:

            # ---- constants / bias tables ----
            tab_t = cpool.tile([128, NTAB], F32)
            nc.sync.dma_start(out=tab_t[:], in_=tabT[:])
            idx_t = cpool.tile([128, NPG // 16], I16)
            nc.sync.dma_start(out=idx_t[:], in_=idx16[:])

            ones_f = cpool.tile([128, 32], F32)
            nc.vector.memset(ones_f[:], 1.0)
            ones_b = cpool.tile([1, 128], BF)
            nc.vector.memset(ones_b[:], 1.0)
            vb_t = cpool.tile([1, C], BF)
            nc.sync.dma_start(out=vb_t[:], in_=vb[:])
            qb_t = cpool.tile([128, 3], F32)
            nc.sync.dma_start(out=qb_t[:], in_=qb_r[:])
            kb_t = cpool.tile([128, 3], F32)
            nc.sync.dma_start(out=kb_t[:], in_=kb_r[:])
            pb_t = cpool.tile([128, 3], F32)
            nc.sync.dma_start(out=pb_t[:], in_=pb_r[:])

            wq = [cpool.tile([128, C], BF, tag=f"wq{k}", name=f"wq{k}") for k in range(3)]
            wk = [cpool.tile([128, C], BF, tag=f"wk{k}", name=f"wk{k}") for k in range(3)]
            wv = [cpool.tile([128, C], BF, tag=f"wv{k}", name=f"wv{k}") for k in range(3)]
            for kt in range(3):
                nc.sync.dma_start(out=wq[kt][:], in_=qwT[kt])
                nc.sync.dma_start(out=wk[kt][:], in_=kwT[kt])
                nc.sync.dma_start(out=wv[kt][:], in_=vwT[kt])
            wp = cpool.tile([32, HEADS * 3 * 128], BF)
            nc.sync.dma_start(out=wp[:], in_=pw4.rearrange("d h m o -> d (h m o)"))

            # ---- bias gather: gath[16g+h, jp*128+i] = table[idx, h] ----
            # chunked by jp-quarters to bound SBUF and pipeline bounce writes
            NCHUNK = 4
            CK = NPG // NCHUNK
            bflat = bounce.rearrange("g c jp i -> (g c) (jp i)")
            for q in range(NCHUNK):
                gath = gpool.tile([128, CK], F32, tag="g", name=f"gath{q}")
                nc.gpsimd.ap_gather(
                    out_ap=gath[:], in_ap=tab_t[:],
                    idxs_ap=idx_t[:, q * (CK // 16):(q + 1) * (CK // 16)],
                    channels=128, num_elems=NTAB, d=1, num_idxs=CK)
                nc.sync.dma_start(out=bflat[:, q * CK:(q + 1) * CK], in_=gath[:])
            # read back j-partitioned per g, exp into expB[p, (h, g, i)]
            expB = cpool.tile([128, HEADS * NJT * NI], BF)
            expB4 = expB[:].rearrange("p (h g i) -> p h g i", h=HEADS, g=NJT)
            exp_insts = []
            for g in range(NJT):
                bfg = gpool.tile([128, HEADS * NI], F32, tag="g", name=f"bfg{g}")
                nc.sync.dma_start(
                    out=bfg[:].rearrange("p (h i) -> p h i", h=HEADS),
                    in_=bounce[g, 0:HEADS].transpose([1, 0, 2]))
                ei = nc.scalar.activation(expB4[:, :, g, :], bfg[:].rearrange(
                    "p (h i) -> p h i", h=HEADS),
                    mybir.ActivationFunctionType.Exp)
                exp_insts.append(ei)

            # ---- per-batch attention ----
            for b in range(nb):
                # load activations
                xb = [wpool.tile([128, N], BF, tag=f"xb{kt}", name=f"xb{b}_{kt}", bufs=2) for kt in range(3)]
                for kt in range(3):
                    nc.sync.dma_start(out=xb[kt][:], in_=xf[b, kt])
                xqb = wpool.tile([128, 3 * NI], BF, tag="xqb", bufs=4)
                nc.sync.dma_start(out=xqb[:].rearrange("c (k i) -> c k i", k=3),
                                  in_=xq[b].transpose([1, 0, 2]))

                # k projection -> kT_sb [mt][128, N] bf16 (+bias)
                kT = [wpool.tile([128, N], BF, tag=f"kT{mt}", name=f"kT{b}_{mt}", bufs=4) for mt in range(3)]
                for mt in range(3):
                    kps = pbig.tile([128, N], F32, tag="big")
                    for half in range(2):
                        for kt in range(3):
                            nc.tensor.matmul(
                                kps[:, half * 512:(half + 1) * 512],
                                lhsT=wk[kt][:, mt * 128:(mt + 1) * 128],
                                rhs=xb[kt][:, half * 512:(half + 1) * 512],
                                start=(kt == 0), stop=(kt == 2))
                    nc.vector.tensor_scalar_add(kT[mt][:], kps[:], kb_t[:, mt:mt + 1])

                # v^T projection -> vt_aug [128, jt, h, 33] bf16, col 32 = ones
                vt = wpool.tile([128, NJT * HEADS * 33], BF, tag="vt", bufs=4)
                vt4 = vt[:].rearrange("p (j h c) -> p j h c", j=NJT, h=HEADS)
                nc.vector.memset(vt[:], 1.0)
                for jt in range(NJT):
                    vps = psmall.tile([128, C], F32, tag="small")
                    for kt in range(3):
                        nc.tensor.matmul(
                            vps[:], lhsT=xb[kt][:, jt * 128:(jt + 1) * 128],
                            rhs=wv[kt][:], start=(kt == 0), stop=False)
                    nc.tensor.matmul(vps[:], lhsT=ones_b[:, :128], rhs=vb_t[:],
                                     start=False, stop=True)
                    nc.vector.tensor_copy(
                        vt4[:, jt, :, 0:32],
                        vps[:].rearrange("p (h c) -> p h c", h=HEADS))

                # q projection -> qT_sb [128, 3, NI] bf16 (+bias)
                qT = wpool.tile([128, 3 * NI], BF, tag="qT", bufs=4)
                for mt in range(3):
                    qps = psmall.tile([128, NI], F32, tag="small")
                    for kt in range(3):
                        nc.tensor.matmul(
                            qps[:], lhsT=wq[kt][:, mt * 128:(mt + 1) * 128],
                            rhs=xqb[:, kt * NI:(kt + 1) * NI],
                            start=(kt == 0), stop=(kt == 2))
                    nc.vector.tensor_scalar_add(
                        qT[:, mt * NI:(mt + 1) * NI], qps[:], qb_t[:, mt:mt + 1])

                # attention per head
                O_all = wpool.tile([32, HEADS * NI], BF, tag="Oall")
                for h in range(HEADS):
                    mt, po = h // 4, 32 * (h % 4)
                    stp = pbig.tile([128, NJT * NI], F32, tag="big")
                    for jt in range(NJT):
                        mm = nc.tensor.matmul(
                            stp[:, jt * NI:(jt + 1) * NI],
                            lhsT=kT[mt][po:po + 32, jt * 128:(jt + 1) * 128],
                            rhs=qT[po:po + 32, mt * NI:(mt + 1) * NI],
                            start=True, stop=True, tile_position=(po, 0))
                        add_dep_helper(mm.ins, exp_insts[-1].ins, sync=False,
                                       reason="hold S^T until expB ready")
                    expS = apool.tile([128, NJT * NI], BF, tag="expS")
                    nc.scalar.activation(expS[:], stp[:],
                                         mybir.ActivationFunctionType.Exp,
                                         scale=float(SCALE))
                    PT = apool.tile([128, NJT * NI], BF, tag="PT")
                    nc.vector.tensor_mul(
                        PT[:], expS[:],
                        expB[:, h * NJT * NI:(h + 1) * NJT * NI])
                    otp = potp.tile([33, NI], F32, tag="otp")
                    for jt in range(NJT):
                        nc.tensor.matmul(
                            otp[:], lhsT=vt4[:, jt, h, :],
                            rhs=PT[:, jt * NI:(jt + 1) * NI],
                            start=(jt == 0), stop=(jt == NJT - 1))
                    rec = apool.tile([128, NI], F32, tag="rec")
                    nc.vector.reciprocal(rec[32:33, :], otp[32:33, :])
                    bcp = pbcp.tile([32, NI], F32, tag="bcp")
                    nc.tensor.matmul(bcp[:], lhsT=ones_f[32:33, :],
                                     rhs=rec[32:33, :], start=True, stop=True,
                                     tile_position=(32, 0))
                    bcs = apool.tile([32, NI], F32, tag="bcs")
                    nc.scalar.activation(bcs[:], bcp[:],
                                         mybir.ActivationFunctionType.Copy)
                    nc.vector.tensor_mul(O_all[:, h * NI:(h + 1) * NI],
                                         otp[0:32, :], bcs[:])

                # output projection
                for mt in range(3):
                    ops = psmall.tile([128, NI], F32, tag="small")
                    for h in range(HEADS):
                        nc.tensor.matmul(
                            ops[:], lhsT=wp[:, (h * 3 + mt) * 128:(h * 3 + mt + 1) * 128],
                            rhs=O_all[:, h * NI:(h + 1) * NI],
                            start=(h == 0), stop=(h == HEADS - 1))
                    osb = wpool.tile([128, NI], F32, tag="osb")
                    nc.vector.tensor_scalar_add(osb[:], ops[:], pb_t[:, mt:mt + 1])
                    nc.sync.dma_start(out=out[b, mt], in_=osb[:])

    nc.compile()
    return nc


def _prep_inputs(x, q_w, q_b, kv_w, kv_b, proj_w, proj_b, rpb_table, rel_index,
                 core, nb=B):
    """Host-side sharding/layout prep for one core (numpy only)."""
    i0 = core * NI
    xfl = x.reshape(B, C, N)[:nb]
    xf = np.ascontiguousarray(xfl.reshape(nb, 3, 128, N)).astype(BF16)
    xq = np.ascontiguousarray(xfl[:, :, i0:i0 + NI].reshape(nb, 3, 128, NI)).astype(BF16)
    qwT = np.ascontiguousarray(q_w.T.reshape(3, 128, C)).astype(BF16)
    kwT = np.ascontiguousarray(kv_w[:C].T.reshape(3, 128, C)).astype(BF16)
    vwT = np.ascontiguousarray(kv_w[C:].T.reshape(3, 128, C)).astype(BF16)
    # pw4[d, h, mt, dout] = proj_w[mt*128+dout, h*32+d]
    pw4 = np.ascontiguousarray(
        proj_w.reshape(3, 128, HEADS, 32).transpose(3, 2, 0, 1)).astype(BF16)
    qb_r = np.ascontiguousarray(q_b.reshape(3, 128).T).astype(np.float32)
    kb_r = np.ascontiguousarray(kv_b[:C].reshape(3, 128).T).astype(np.float32)
    pb_r = np.ascontiguousarray(proj_b.reshape(3, 128).T).astype(np.float32)
    vb = np.ascontiguousarray(kv_b[C:].reshape(1, C)).astype(BF16)
    tabT = np.zeros((128, NTAB), np.float32)
    for c in range(128):
        if c % 16 < HEADS:
            tabT[c] = rpb_table[:, c % 16]
    # gather cell list: group g = j-tile; slot k = jp*128 + i
    # value = rel_index[i0+i, g*128+jp]  (bias^T[j, i] = table[rel_index[i, j]])
    ri = rel_index[i0:i0 + NI]  # [NI, N] int32
    idx16 = np.zeros((128, 1024), np.int16)
    for g in range(NJT):
        blk = ri[:, g * 128:(g + 1) * 128].T  # [jp, i]
        lst = blk.reshape(-1).astype(np.int16)  # k = jp*128+i
        idx16[16 * g:16 * g + 16, :] = lst.reshape(-1, 16).T
    return {"xf": xf, "xq": xq, "qwT": qwT, "kwT": kwT, "vwT": vwT, "pw4": pw4,
            "qb_r": qb_r, "kb_r": kb_r, "pb_r": pb_r, "vb": vb,
            "tabT": tabT, "idx16": idx16}


def kernel(x, q_w, q_b, kv_w, kv_b, proj_w, proj_b, rpb_table, rel_index):
    x = np.asarray(x, np.float32)
    q_w = np.asarray(q_w, np.float32); q_b = np.asarray(q_b, np.float32)
    kv_w = np.asarray(kv_w, np.float32); kv_b = np.asarray(kv_b, np.float32)
    proj_w = np.asarray(proj_w, np.float32); proj_b = np.asarray(proj_b, np.float32)
    rpb_table = np.asarray(rpb_table, np.float32)
    rel_index = np.asarray(rel_index, np.int32)

    if "nc" not in _CACHE:
        _CACHE["nc"] = build_nc(B)
    nc = _CACHE["nc"]
    in_maps = [
        _prep_inputs(x, q_w, q_b, kv_w, kv_b, proj_w, proj_b, rpb_table,
                     rel_index, core)
        for core in range(NCORES)
    ]
    res = bass_utils.run_bass_kernel_spmd(nc, in_maps, list(range(NCORES)),
                                          trace=False)
    out = np.empty((B, C, N), np.float32)
    for core in range(NCORES):
        o = res.results[core]["out"]  # [B, 3, 128, NI]
        out[:, :, core * NI:(core + 1) * NI] = o.reshape(B, C, NI)
    return out.reshape(B, C, Hh, Ww)


# revision 14
# speedup vs baseline: 1.0270x; 1.0270x over previous
"""Trainium2 Bass kernel for nn_Attention_60224031424957.

Attention block: x[8,384,32,32] -> 1x1 conv QKV -> 12-head attention with
relative-position bias (rpb_table[rel_index]) -> softmax -> proj.

Sharding: i-block data parallel. Core c owns query columns i in
[128c, 128c+128) for ALL batches/heads; k/v are computed per-core for all
batches (duplicated). The bias gather (the dominant cost: 131072 random
table rows per core) runs on GPSIMD via ap_gather with one head per
channel, then a DRAM bounce rearranges it to j-partitioned planes.

Layouts on device (per core):
  S^T[j, i] tiles: psum [128 j-in-tile, (jt,i)] per head; softmax over j via
  a ones-column appended to V (row 32 of the O^T psum = denominators).
  P^T = exp(scale*S^T) (*) exp(biasT)   (bf16)
  O^T = V_aug^T @ P^T accumulated over j-tiles; normalized by 1/sums.
"""

import numpy as np
import ml_dtypes

import concourse.bacc as bacc
import concourse.mybir as mybir
import concourse.tile as tile
from concourse.tile import add_dep_helper
from concourse import bass_utils
from concourse._compat import get_trn_type

BF16 = ml_dtypes.bfloat16

B, C, Hh, Ww = 8, 384, 32, 32
N = Hh * Ww            # 1024
HEADS = 12
HD = 32                # head dim
NTAB = (2 * Hh - 1) * (2 * Ww - 1)  # 3969
NCORES = 8
NI = N // NCORES       # 128 query columns per core
NJT = 8                # j tiles of 128
SCALE = HD ** -0.5

F32 = mybir.dt.float32
BF = mybir.dt.bfloat16
I16 = mybir.dt.int16

_CACHE = {}


def build_nc(nb=B):
    nc = bacc.Bacc(get_trn_type() or "TRN2", target_bir_lowering=False, debug=False)

    # ---- DRAM inputs ----
    xf = nc.dram_tensor("xf", [nb, 3, 128, N], BF, kind="ExternalInput")      # full x, [b, kt, c_in_tile, n]
    xq = nc.dram_tensor("xq", [nb, 3, 128, NI], BF, kind="ExternalInput")     # per-core x slice for q
    qwT = nc.dram_tensor("qwT", [3, 128, C], BF, kind="ExternalInput")        # q_w.T tiles [kt, c, dout]
    kwT = nc.dram_tensor("kwT", [3, 128, C], BF, kind="ExternalInput")
    vwT = nc.dram_tensor("vwT", [3, 128, C], BF, kind="ExternalInput")
    pw4 = nc.dram_tensor("pw4", [32, HEADS, 3, 128], BF, kind="ExternalInput")  # proj_w [d, h, mt, dout]
    qb_r = nc.dram_tensor("qb_r", [128, 3], F32, kind="ExternalInput")
    kb_r = nc.dram_tensor("kb_r", [128, 3], F32, kind="ExternalInput")
    pb_r = nc.dram_tensor("pb_r", [128, 3], F32, kind="ExternalInput")
    vb = nc.dram_tensor("vb", [1, C], BF, kind="ExternalInput")               # kv_b[384:]
    tabT = nc.dram_tensor("tabT", [128, NTAB], F32, kind="ExternalInput")     # table col h at chan 16g+h
    idx16 = nc.dram_tensor("idx16", [128, 1024], I16, kind="ExternalInput")
    # idx16: [128, 1024] int16, group g rows 16g..16g+15, wrapped per-16

    out = nc.dram_tensor("out", [nb, 3, 128, NI], F32, kind="ExternalOutput")
    bounce = nc.dram_tensor("bounce", [NJT, 16, 128, NI], F32)  # [g, chan16, jp, i]

    NCELL = NJT * 128 * NI      # 131072 cells per core
    NPG = NCELL // NJT          # 16384 per group

    with tile.TileContext(nc) as tc:
        with tc.tile_pool(name="const", bufs=1) as cpool, \
             tc.tile_pool(name="gbuf", bufs=1) as gpool, \
             tc.tile_pool(name="work", bufs=2) as wpool, \
             tc.tile_pool(name="attn", bufs=3) as apool, \
             tc.tile_pool(name="pbig", bufs=4, space="PSUM") as pbig, \
             tc.tile_pool(name="potp", bufs=2, space="PSUM") as potp, \
             tc.tile_pool(name="pbcp", bufs=1, space="PSUM") as pbcp, \
             tc.tile_pool(name="psmall", bufs=1, space="PSUM") as psmall:

            # ---- constants / bias tables ----
            tab_t = cpool.tile([128, NTAB], F32)
            nc.sync.dma_start(out=tab_t[:], in_=tabT[:])
            idx_t = cpool.tile([128, NPG // 16], I16)
            nc.sync.dma_start(out=idx_t[:], in_=idx16[:])

            ones_f = cpool.tile([128, 32], F32)
            nc.vector.memset(ones_f[:], 1.0)
            ones_b = cpool.tile([1, 128], BF)
            nc.vector.memset(ones_b[:], 1.0)
            vb_t = cpool.tile([1, C], BF)
            nc.sync.dma_start(out=vb_t[:], in_=vb[:])
            qb_t = cpool.tile([128, 3], F32)
            nc.sync.dma_start(out=qb_t[:], in_=qb_r[:])
            kb_t = cpool.tile([128, 3], F32)
            nc.sync.dma_start(out=kb_t[:], in_=kb_r[:])
            pb_t = cpool.tile([128, 3], F32)
            nc.sync.dma_start(out=pb_t[:], in_=pb_r[:])

            wq = [cpool.tile([128, C], BF, tag=f"wq{k}", name=f"wq{k}") for k in range(3)]
            wk = [cpool.tile([128, C], BF, tag=f"wk{k}", name=f"wk{k}") for k in range(3)]
            wv = [cpool.tile([128, C], BF, tag=f"wv{k}", name=f"wv{k}") for k in range(3)]
            for kt in range(3):
                nc.sync.dma_start(out=wq[kt][:], in_=qwT[kt])
                nc.sync.dma_start(out=wk[kt][:], in_=kwT[kt])
                nc.sync.dma_start(out=wv[kt][:], in_=vwT[kt])
            wp = cpool.tile([32, HEADS * 3 * 128], BF)
            nc.sync.dma_start(out=wp[:], in_=pw4.rearrange("d h m o -> d (h m o)"))

            # ---- bias gather: gath[16g+h, jp*128+i] = table[idx, h] ----
            # chunked by jp-quarters to bound SBUF and pipeline bounce writes
            NCHUNK = 4
            CK = NPG // NCHUNK
            bflat = bounce.rearrange("g c jp i -> (g c) (jp i)")
            for q in range(NCHUNK):
                gath = gpool.tile([128, CK], F32, tag="g", name=f"gath{q}")
                nc.gpsimd.ap_gather(
                    out_ap=gath[:], in_ap=tab_t[:],
                    idxs_ap=idx_t[:, q * (CK // 16):(q + 1) * (CK // 16)],
                    channels=128, num_elems=NTAB, d=1, num_idxs=CK)
                nc.scalar.dma_start(out=bflat[:, q * CK:(q + 1) * CK], in_=gath[:])
            # read back j-partitioned per g, exp into expB[p, (h, g, i)]
            expB = cpool.tile([128, HEADS * NJT * NI], BF)
            expB4 = expB[:].rearrange("p (h g i) -> p h g i", h=HEADS, g=NJT)
            exp_insts = []
            for g in range(NJT):
                bfg = gpool.tile([128, HEADS * NI], F32, tag="g", name=f"bfg{g}")
                nc.scalar.dma_start(
                    out=bfg[:].rearrange("p (h i) -> p h i", h=HEADS),
                    in_=bounce[g, 0:HEADS].transpose([1, 0, 2]))
                ei = nc.scalar.activation(expB4[:, :, g, :], bfg[:].rearrange(
                    "p (h i) -> p h i", h=HEADS),
                    mybir.ActivationFunctionType.Exp)
                exp_insts.append(ei)

            # ---- per-batch attention ----
            for b in range(nb):
                # load activations
                xb = [wpool.tile([128, N], BF, tag=f"xb{kt}", name=f"xb{b}_{kt}", bufs=2) for kt in range(3)]
                for kt in range(3):
                    nc.sync.dma_start(out=xb[kt][:], in_=xf[b, kt])
                xqb = wpool.tile([128, 3 * NI], BF, tag="xqb", bufs=4)
                nc.sync.dma_start(out=xqb[:].rearrange("c (k i) -> c k i", k=3),
                                  in_=xq[b].transpose([1, 0, 2]))

                # k projection -> kT_sb [mt][128, N] bf16 (+bias)
                kT = [wpool.tile([128, N], BF, tag=f"kT{mt}", name=f"kT{b}_{mt}", bufs=4) for mt in range(3)]
                for mt in range(3):
                    for half in range(2):
                        kps = pbig.tile([128, 512], F32, tag="big",
                                        name=f"kps{b}_{mt}_{half}")
                        for kt in range(3):
                            nc.tensor.matmul(
                                kps[:],
                                lhsT=wk[kt][:, mt * 128:(mt + 1) * 128],
                                rhs=xb[kt][:, half * 512:(half + 1) * 512],
                                start=(kt == 0), stop=(kt == 2))
                        nc.vector.tensor_scalar_add(
                            kT[mt][:, half * 512:(half + 1) * 512], kps[:],
                            kb_t[:, mt:mt + 1])

                # v^T projection -> vt_aug [128, jt, h, 33] bf16, col 32 = ones
                vt = wpool.tile([128, NJT * HEADS * 33], BF, tag="vt", bufs=4)
                vt4 = vt[:].rearrange("p (j h c) -> p j h c", j=NJT, h=HEADS)
                nc.vector.memset(vt[:], 1.0)
                for jt in range(NJT):
                    vps = psmall.tile([128, C], F32, tag="small")
                    for kt in range(3):
                        nc.tensor.matmul(
                            vps[:], lhsT=xb[kt][:, jt * 128:(jt + 1) * 128],
                            rhs=wv[kt][:], start=(kt == 0), stop=False)
                    nc.tensor.matmul(vps[:], lhsT=ones_b[:, :128], rhs=vb_t[:],
                                     start=False, stop=True)
                    nc.vector.tensor_copy(
                        vt4[:, jt, :, 0:32],
                        vps[:].rearrange("p (h c) -> p h c", h=HEADS))

                # q projection -> qT_sb [128, 3, NI] bf16 (+bias)
                qT = wpool.tile([128, 3 * NI], BF, tag="qT", bufs=4)
                for mt in range(3):
                    qps = psmall.tile([128, NI], F32, tag="small")
                    for kt in range(3):
                        nc.tensor.matmul(
                            qps[:], lhsT=wq[kt][:, mt * 128:(mt + 1) * 128],
                            rhs=xqb[:, kt * NI:(kt + 1) * NI],
                            start=(kt == 0), stop=(kt == 2))
                    nc.vector.tensor_scalar_add(
                        qT[:, mt * NI:(mt + 1) * NI], qps[:], qb_t[:, mt:mt + 1])

                # attention per head
                O_all = wpool.tile([32, HEADS * NI], BF, tag="Oall")
                for h in range(HEADS):
                    mt, po = h // 4, 32 * (h % 4)
                    expS = apool.tile([128, NJT * NI], BF, tag="expS")
                    for half in range(2):
                        stp = pbig.tile([128, 512], F32, tag="big",
                                        name=f"stp{b}_{h}_{half}")
                        for jj in range(4):
                            jt = half * 4 + jj
                            mm = nc.tensor.matmul(
                                stp[:, jj * NI:(jj + 1) * NI],
                                lhsT=kT[mt][po:po + 32, jt * 128:(jt + 1) * 128],
                                rhs=qT[po:po + 32, mt * NI:(mt + 1) * NI],
                                start=True, stop=True, tile_position=(po, 0))
                            add_dep_helper(mm.ins, exp_insts[-1].ins, sync=False,
                                           reason="hold S^T until expB ready")
                        nc.scalar.activation(
                            expS[:, half * 512:(half + 1) * 512], stp[:],
                            mybir.ActivationFunctionType.Exp,
                            scale=float(SCALE))
                    PT = apool.tile([128, NJT * NI], BF, tag="PT")
                    nc.vector.tensor_mul(
                        PT[:], expS[:],
                        expB[:, h * NJT * NI:(h + 1) * NJT * NI])
                    otp = potp.tile([33, NI], F32, tag="otp")
                    for jt in range(NJT):
                        nc.tensor.matmul(
                            otp[:], lhsT=vt4[:, jt, h, :],
                            rhs=PT[:, jt * NI:(jt + 1) * NI],
                            start=(jt == 0), stop=(jt == NJT - 1))
                    rec = apool.tile([128, NI], F32, tag="rec")
                    nc.vector.reciprocal(rec[32:33, :], otp[32:33, :])
                    bcp = pbcp.tile([32, NI], F32, tag="bcp")
                    nc.tensor.matmul(bcp[:], lhsT=ones_f[32:33, :],
                                     rhs=rec[32:33, :], start=True, stop=True,
                                     tile_position=(32, 0))
                    bcs = apool.tile([32, NI], F32, tag="bcs")
                    nc.scalar.activation(bcs[:], bcp[:],
                                         mybir.ActivationFunctionType.Copy)
                    nc.vector.tensor_mul(O_all[:, h * NI:(h + 1) * NI],
                                         otp[0:32, :], bcs[:])

                # output projection
                for mt in range(3):
                    ops = psmall.tile([128, NI], F32, tag="small")
                    for h in range(HEADS):
                        nc.tensor.matmul(
                            ops[:], lhsT=wp[:, (h * 3 + mt) * 128:(h * 3 + mt + 1) * 128],
                            rhs=O_all[:, h * NI:(h + 1) * NI],
                            start=(h == 0), stop=(h == HEADS - 1))
                    osb = wpool.tile([128, NI], F32, tag="osb")
                    nc.vector.tensor_scalar_add(osb[:], ops[:], pb_t[:, mt:mt + 1])
                    nc.sync.dma_start(out=out[b, mt], in_=osb[:])

    nc.compile()
    return nc


def _prep_inputs(x, q_w, q_b, kv_w, kv_b, proj_w, proj_b, rpb_table, rel_index,
                 core, nb=B):
    """Host-side sharding/layout prep for one core (numpy only)."""
    i0 = core * NI
    xfl = x.reshape(B, C, N)[:nb]
    xf = np.ascontiguousarray(xfl.reshape(nb, 3, 128, N)).astype(BF16)
    xq = np.ascontiguousarray(xfl[:, :, i0:i0 + NI].reshape(nb, 3, 128, NI)).astype(BF16)
    qwT = np.ascontiguousarray(q_w.T.reshape(3, 128, C)).astype(BF16)
    kwT = np.ascontiguousarray(kv_w[:C].T.reshape(3, 128, C)).astype(BF16)
    vwT = np.ascontiguousarray(kv_w[C:].T.reshape(3, 128, C)).astype(BF16)
    # pw4[d, h, mt, dout] = proj_w[mt*128+dout, h*32+d]
    pw4 = np.ascontiguousarray(
        proj_w.reshape(3, 128, HEADS, 32).transpose(3, 2, 0, 1)).astype(BF16)
    qb_r = np.ascontiguousarray(q_b.reshape(3, 128).T).astype(np.float32)
    kb_r = np.ascontiguousarray(kv_b[:C].reshape(3, 128).T).astype(np.float32)
    pb_r = np.ascontiguousarray(proj_b.reshape(3, 128).T).astype(np.float32)
    vb = np.ascontiguousarray(kv_b[C:].reshape(1, C)).astype(BF16)
    tabT = np.zeros((128, NTAB), np.float32)
    for c in range(128):
        if c % 16 < HEADS:
            tabT[c] = rpb_table[:, c % 16]
    # gather cell list: group g = j-tile; slot k = jp*128 + i
    # value = rel_index[i0+i, g*128+jp]  (bias^T[j, i] = table[rel_index[i, j]])
    ri = rel_index[i0:i0 + NI]  # [NI, N] int32
    idx16 = np.zeros((128, 1024), np.int16)
    for g in range(NJT):
        blk = ri[:, g * 128:(g + 1) * 128].T  # [jp, i]
        lst = blk.reshape(-1).astype(np.int16)  # k = jp*128+i
        idx16[16 * g:16 * g + 16, :] = lst.reshape(-1, 16).T
    return {"xf": xf, "xq": xq, "qwT": qwT, "kwT": kwT, "vwT": vwT, "pw4": pw4,
            "qb_r": qb_r, "kb_r": kb_r, "pb_r": pb_r, "vb": vb,
            "tabT": tabT, "idx16": idx16}


def kernel(x, q_w, q_b, kv_w, kv_b, proj_w, proj_b, rpb_table, rel_index):
    x = np.asarray(x, np.float32)
    q_w = np.asarray(q_w, np.float32); q_b = np.asarray(q_b, np.float32)
    kv_w = np.asarray(kv_w, np.float32); kv_b = np.asarray(kv_b, np.float32)
    proj_w = np.asarray(proj_w, np.float32); proj_b = np.asarray(proj_b, np.float32)
    rpb_table = np.asarray(rpb_table, np.float32)
    rel_index = np.asarray(rel_index, np.int32)

    if "nc" not in _CACHE:
        _CACHE["nc"] = build_nc(B)
    nc = _CACHE["nc"]
    in_maps = [
        _prep_inputs(x, q_w, q_b, kv_w, kv_b, proj_w, proj_b, rpb_table,
                     rel_index, core)
        for core in range(NCORES)
    ]
    res = bass_utils.run_bass_kernel_spmd(nc, in_maps, list(range(NCORES)),
                                          trace=False)
    out = np.empty((B, C, N), np.float32)
    for core in range(NCORES):
        o = res.results[core]["out"]  # [B, 3, 128, NI]
        out[:, :, core * NI:(core + 1) * NI] = o.reshape(B, C, NI)
    return out.reshape(B, C, Hh, Ww)


# revision 16
# speedup vs baseline: 1.2213x; 1.1891x over previous
"""Trainium2 Bass kernel for nn_Attention_60224031424957.

Attention block: x[8,384,32,32] -> 1x1 conv QKV -> 12-head attention with
relative-position bias (rpb_table[rel_index]) -> softmax -> proj.

Sharding: i-block data parallel. Core c owns query columns i in
[128c, 128c+128) for ALL batches/heads; k/v are computed per-core for all
batches (duplicated). The bias gather (the dominant cost: 131072 random
table rows per core) runs on GPSIMD via ap_gather with one head per
channel, then a DRAM bounce rearranges it to j-partitioned planes.

Layouts on device (per core):
  S^T[j, i] tiles: psum [128 j-in-tile, (jt,i)] per head; softmax over j via
  a ones-column appended to V (row 32 of the O^T psum = denominators).
  P^T = exp(scale*S^T) (*) exp(biasT)   (bf16)
  O^T = V_aug^T @ P^T accumulated over j-tiles; normalized by 1/sums.
"""

import numpy as np
import ml_dtypes

import concourse.bacc as bacc
import concourse.mybir as mybir
import concourse.tile as tile
from concourse.tile import add_dep_helper
from concourse import bass_utils
from concourse._compat import get_trn_type

BF16 = ml_dtypes.bfloat16

B, C, Hh, Ww = 8, 384, 32, 32
N = Hh * Ww            # 1024
HEADS = 12
HD = 32                # head dim
NTAB = (2 * Hh - 1) * (2 * Ww - 1)  # 3969
NCORES = 8
NI = N // NCORES       # 128 query columns per core
NJT = 8                # j tiles of 128
SCALE = HD ** -0.5

F32 = mybir.dt.float32
BF = mybir.dt.bfloat16
I16 = mybir.dt.int16

_CACHE = {}


def build_nc(nb=B):
    nc = bacc.Bacc(get_trn_type() or "TRN2", target_bir_lowering=False, debug=False)

    # ---- DRAM inputs ----
    xf = nc.dram_tensor("xf", [nb, 3, 128, N], BF, kind="ExternalInput")      # full x, [b, kt, c_in_tile, n]
    xq = nc.dram_tensor("xq", [nb, 3, 128, NI], BF, kind="ExternalInput")     # per-core x slice for q
    qwT = nc.dram_tensor("qwT", [3, 128, C], BF, kind="ExternalInput")        # q_w.T tiles [kt, c, dout]
    kwT = nc.dram_tensor("kwT", [3, 128, C], BF, kind="ExternalInput")
    vwT = nc.dram_tensor("vwT", [3, 128, C], BF, kind="ExternalInput")
    pw4 = nc.dram_tensor("pw4", [32, HEADS, 3, 128], BF, kind="ExternalInput")  # proj_w [d, h, mt, dout]
    qb_r = nc.dram_tensor("qb_r", [128, 3], F32, kind="ExternalInput")
    kb_r = nc.dram_tensor("kb_r", [128, 3], F32, kind="ExternalInput")
    pb_r = nc.dram_tensor("pb_r", [128, 3], F32, kind="ExternalInput")
    vb = nc.dram_tensor("vb", [1, C], BF, kind="ExternalInput")               # kv_b[384:]
    tabT = nc.dram_tensor("tabT", [128, NTAB], F32, kind="ExternalInput")     # table col h at chan 16g+h
    idx16 = nc.dram_tensor("idx16", [128, 1024], I16, kind="ExternalInput")
    # idx16: [128, 1024] int16, group g rows 16g..16g+15, wrapped per-16

    out = nc.dram_tensor("out", [nb, 3, 128, NI], F32, kind="ExternalOutput")
    bounce = nc.dram_tensor("bounce", [NJT, 16, 128, NI], F32)  # [g, chan16, jp, i]

    NCELL = NJT * 128 * NI      # 131072 cells per core
    NPG = NCELL // NJT          # 16384 per group

    with tile.TileContext(nc) as tc:
        with tc.tile_pool(name="const", bufs=1) as cpool, \
             tc.tile_pool(name="gbuf", bufs=1) as gpool, \
             tc.tile_pool(name="work", bufs=2) as wpool, \
             tc.tile_pool(name="attn", bufs=2) as apool, \
             tc.tile_pool(name="pbig", bufs=4, space="PSUM") as pbig, \
             tc.tile_pool(name="potp", bufs=2, space="PSUM") as potp, \
             tc.tile_pool(name="pbcp", bufs=1, space="PSUM") as pbcp, \
             tc.tile_pool(name="psmall", bufs=1, space="PSUM") as psmall:

            # ---- constants / bias tables ----
            tab_t = cpool.tile([128, NTAB], F32)
            nc.sync.dma_start(out=tab_t[:], in_=tabT[:])
            idx_t = cpool.tile([128, NPG // 16], I16)
            nc.sync.dma_start(out=idx_t[:], in_=idx16[:])

            ones_f = cpool.tile([128, 32], F32)
            nc.vector.memset(ones_f[:], 1.0)
            ones_b = cpool.tile([1, 128], BF)
            nc.vector.memset(ones_b[:], 1.0)
            vb_t = cpool.tile([1, C], BF)
            nc.sync.dma_start(out=vb_t[:], in_=vb[:])
            qb_t = cpool.tile([128, 3], F32)
            nc.sync.dma_start(out=qb_t[:], in_=qb_r[:])
            kb_t = cpool.tile([128, 3], F32)
            nc.sync.dma_start(out=kb_t[:], in_=kb_r[:])
            pb_t = cpool.tile([128, 3], F32)
            nc.sync.dma_start(out=pb_t[:], in_=pb_r[:])

            wq = [cpool.tile([128, C], BF, tag=f"wq{k}", name=f"wq{k}") for k in range(3)]
            wk = [cpool.tile([128, C], BF, tag=f"wk{k}", name=f"wk{k}") for k in range(3)]
            wv = [cpool.tile([128, C], BF, tag=f"wv{k}", name=f"wv{k}") for k in range(3)]
            for kt in range(3):
                nc.sync.dma_start(out=wq[kt][:], in_=qwT[kt])
                nc.sync.dma_start(out=wk[kt][:], in_=kwT[kt])
                nc.sync.dma_start(out=wv[kt][:], in_=vwT[kt])
            wp = cpool.tile([32, HEADS * 3 * 128], BF)
            nc.sync.dma_start(out=wp[:], in_=pw4.rearrange("d h m o -> d (h m o)"))

            # ---- bias gather: gath[16g+h, jp*128+i] = table[idx, h] ----
            # chunked by jp-quarters to bound SBUF and pipeline bounce writes
            NCHUNK = 4
            CK = NPG // NCHUNK
            bflat = bounce.rearrange("g c jp i -> (g c) (jp i)")
            for q in range(NCHUNK):
                gath = gpool.tile([128, CK], F32, tag="g", name=f"gath{q}")
                nc.gpsimd.ap_gather(
                    out_ap=gath[:], in_ap=tab_t[:],
                    idxs_ap=idx_t[:, q * (CK // 16):(q + 1) * (CK // 16)],
                    channels=128, num_elems=NTAB, d=1, num_idxs=CK)
                nc.scalar.dma_start(out=bflat[:, q * CK:(q + 1) * CK], in_=gath[:])
            # read back j-partitioned per g, exp into expB[p, (h, g, i)]
            expB = cpool.tile([128, HEADS * NJT * NI], BF)
            expB4 = expB[:].rearrange("p (h g i) -> p h g i", h=HEADS, g=NJT)
            exp_insts = []
            for g in range(NJT):
                bfg = gpool.tile([128, HEADS * NI], F32, tag="g", name=f"bfg{g}")
                nc.scalar.dma_start(
                    out=bfg[:].rearrange("p (h i) -> p h i", h=HEADS),
                    in_=bounce[g, 0:HEADS].transpose([1, 0, 2]))
                ei = nc.scalar.activation(expB4[:, :, g, :], bfg[:].rearrange(
                    "p (h i) -> p h i", h=HEADS),
                    mybir.ActivationFunctionType.Exp)
                exp_insts.append(ei)

            # ---- PASS A: projections for all batches (overlaps gather) ----
            kT_all, vt4_all, qT_all = [], [], []
            for b in range(nb):
                xb = [wpool.tile([128, N], BF, tag=f"xb{kt}", name=f"xb{b}_{kt}", bufs=2) for kt in range(3)]
                for kt in range(3):
                    nc.sync.dma_start(out=xb[kt][:], in_=xf[b, kt])
                xqb = wpool.tile([128, 3 * NI], BF, tag="xqb", bufs=1)
                nc.sync.dma_start(out=xqb[:].rearrange("c (k i) -> c k i", k=3),
                                  in_=xq[b].transpose([1, 0, 2]))

                kT = [wpool.tile([128, N], BF, tag=f"kT{mt}", name=f"kT{b}_{mt}", bufs=nb) for mt in range(3)]
                for mt in range(3):
                    for half in range(2):
                        kps = pbig.tile([128, 512], F32, tag="big",
                                        name=f"kps{b}_{mt}_{half}")
                        for kt in range(3):
                            nc.tensor.matmul(
                                kps[:],
                                lhsT=wk[kt][:, mt * 128:(mt + 1) * 128],
                                rhs=xb[kt][:, half * 512:(half + 1) * 512],
                                start=(kt == 0), stop=(kt == 2))
                        nc.vector.tensor_scalar_add(
                            kT[mt][:, half * 512:(half + 1) * 512], kps[:],
                            kb_t[:, mt:mt + 1])

                vt = wpool.tile([128, NJT * HEADS * 33], BF, tag="vt",
                                name=f"vt{b}", bufs=nb)
                vt4 = vt[:].rearrange("p (j h c) -> p j h c", j=NJT, h=HEADS)
                nc.vector.memset(vt[:], 1.0)
                for jt in range(NJT):
                    vps = psmall.tile([128, C], F32, tag="small",
                                      name=f"vps{b}_{jt}")
                    for kt in range(3):
                        nc.tensor.matmul(
                            vps[:], lhsT=xb[kt][:, jt * 128:(jt + 1) * 128],
                            rhs=wv[kt][:], start=(kt == 0), stop=False)
                    nc.tensor.matmul(vps[:], lhsT=ones_b[:, :128], rhs=vb_t[:],
                                     start=False, stop=True)
                    nc.vector.tensor_copy(
                        vt4[:, jt, :, 0:32],
                        vps[:].rearrange("p (h c) -> p h c", h=HEADS))

                qT = wpool.tile([128, 3 * NI], BF, tag="qT", name=f"qT{b}",
                                bufs=nb)
                for mt in range(3):
                    qps = psmall.tile([128, NI], F32, tag="small",
                                      name=f"qps{b}_{mt}")
                    for kt in range(3):
                        nc.tensor.matmul(
                            qps[:], lhsT=wq[kt][:, mt * 128:(mt + 1) * 128],
                            rhs=xqb[:, kt * NI:(kt + 1) * NI],
                            start=(kt == 0), stop=(kt == 2))
                    nc.vector.tensor_scalar_add(
                        qT[:, mt * NI:(mt + 1) * NI], qps[:], qb_t[:, mt:mt + 1])
                kT_all.append(kT); vt4_all.append(vt4); qT_all.append(qT)

            # ---- PASS B: attention per (batch, head) ----
            for b in range(nb):
                kT, vt4, qT = kT_all[b], vt4_all[b], qT_all[b]
                O_all = wpool.tile([32, HEADS * NI], BF, tag="Oall",
                                   name=f"Oall{b}", bufs=2)
                for h in range(HEADS):
                    mt, po = h // 4, 32 * (h % 4)
                    expS = apool.tile([128, NJT * NI], BF, tag="expS")
                    for half in range(2):
                        stp = pbig.tile([128, 512], F32, tag="big",
                                        name=f"stp{b}_{h}_{half}")
                        for jj in range(4):
                            jt = half * 4 + jj
                            nc.tensor.matmul(
                                stp[:, jj * NI:(jj + 1) * NI],
                                lhsT=kT[mt][po:po + 32, jt * 128:(jt + 1) * 128],
                                rhs=qT[po:po + 32, mt * NI:(mt + 1) * NI],
                                start=True, stop=True, tile_position=(po, 0))
                        nc.scalar.activation(
                            expS[:, half * 512:(half + 1) * 512], stp[:],
                            mybir.ActivationFunctionType.Exp,
                            scale=float(SCALE))
                    PT = apool.tile([128, NJT * NI], BF, tag="PT")
                    nc.vector.tensor_mul(
                        PT[:], expS[:],
                        expB[:, h * NJT * NI:(h + 1) * NJT * NI])
                    otp = potp.tile([33, NI], F32, tag="otp")
                    for jt in range(NJT):
                        nc.tensor.matmul(
                            otp[:], lhsT=vt4[:, jt, h, :],
                            rhs=PT[:, jt * NI:(jt + 1) * NI],
                            start=(jt == 0), stop=(jt == NJT - 1))
                    rec = apool.tile([128, NI], F32, tag="rec")
                    nc.vector.reciprocal(rec[32:33, :], otp[32:33, :])
                    bcp = pbcp.tile([32, NI], F32, tag="bcp")
                    nc.tensor.matmul(bcp[:], lhsT=ones_f[32:33, :],
                                     rhs=rec[32:33, :], start=True, stop=True,
                                     tile_position=(32, 0))
                    bcs = apool.tile([32, NI], F32, tag="bcs")
                    nc.scalar.activation(bcs[:], bcp[:],
                                         mybir.ActivationFunctionType.Copy)
                    nc.vector.tensor_mul(O_all[:, h * NI:(h + 1) * NI],
                                         otp[0:32, :], bcs[:])

                for mt in range(3):
                    ops = psmall.tile([128, NI], F32, tag="small",
                                      name=f"ops{b}_{mt}")
                    for h in range(HEADS):
                        nc.tensor.matmul(
                            ops[:], lhsT=wp[:, (h * 3 + mt) * 128:(h * 3 + mt + 1) * 128],
                            rhs=O_all[:, h * NI:(h + 1) * NI],
                            start=(h == 0), stop=(h == HEADS - 1))
                    osb = wpool.tile([128, NI], F32, tag="osb", name=f"osb{b}_{mt}",
                                     bufs=2)
                    nc.vector.tensor_scalar_add(osb[:], ops[:], pb_t[:, mt:mt + 1])
                    nc.sync.dma_start(out=out[b, mt], in_=osb[:])

    nc.compile()
    return nc


def _prep_inputs(x, q_w, q_b, kv_w, kv_b, proj_w, proj_b, rpb_table, rel_index,
                 core, nb=B):
    """Host-side sharding/layout prep for one core (numpy only)."""
    i0 = core * NI
    xfl = x.reshape(B, C, N)[:nb]
    xf = np.ascontiguousarray(xfl.reshape(nb, 3, 128, N)).astype(BF16)
    xq = np.ascontiguousarray(xfl[:, :, i0:i0 + NI].reshape(nb, 3, 128, NI)).astype(BF16)
    qwT = np.ascontiguousarray(q_w.T.reshape(3, 128, C)).astype(BF16)
    kwT = np.ascontiguousarray(kv_w[:C].T.reshape(3, 128, C)).astype(BF16)
    vwT = np.ascontiguousarray(kv_w[C:].T.reshape(3, 128, C)).astype(BF16)
    # pw4[d, h, mt, dout] = proj_w[mt*128+dout, h*32+d]
    pw4 = np.ascontiguousarray(
        proj_w.reshape(3, 128, HEADS, 32).transpose(3, 2, 0, 1)).astype(BF16)
    qb_r = np.ascontiguousarray(q_b.reshape(3, 128).T).astype(np.float32)
    kb_r = np.ascontiguousarray(kv_b[:C].reshape(3, 128).T).astype(np.float32)
    pb_r = np.ascontiguousarray(proj_b.reshape(3, 128).T).astype(np.float32)
    vb = np.ascontiguousarray(kv_b[C:].reshape(1, C)).astype(BF16)
    tabT = np.zeros((128, NTAB), np.float32)
    for c in range(128):
        if c % 16 < HEADS:
            tabT[c] = rpb_table[:, c % 16]
    # gather cell list: group g = j-tile; slot k = jp*128 + i
    # value = rel_index[i0+i, g*128+jp]  (bias^T[j, i] = table[rel_index[i, j]])
    ri = rel_index[i0:i0 + NI]  # [NI, N] int32
    idx16 = np.zeros((128, 1024), np.int16)
    for g in range(NJT):
        blk = ri[:, g * 128:(g + 1) * 128].T  # [jp, i]
        lst = blk.reshape(-1).astype(np.int16)  # k = jp*128+i
        idx16[16 * g:16 * g + 16, :] = lst.reshape(-1, 16).T
    return {"xf": xf, "xq": xq, "qwT": qwT, "kwT": kwT, "vwT": vwT, "pw4": pw4,
            "qb_r": qb_r, "kb_r": kb_r, "pb_r": pb_r, "vb": vb,
            "tabT": tabT, "idx16": idx16}


def kernel(x, q_w, q_b, kv_w, kv_b, proj_w, proj_b, rpb_table, rel_index):
    x = np.asarray(x, np.float32)
    q_w = np.asarray(q_w, np.float32); q_b = np.asarray(q_b, np.float32)
    kv_w = np.asarray(kv_w, np.float32); kv_b = np.asarray(kv_b, np.float32)
    proj_w = np.asarray(proj_w, np.float32); proj_b = np.asarray(proj_b, np.float32)
    rpb_table = np.asarray(rpb_table, np.float32)
    rel_index = np.asarray(rel_index, np.int32)

    if "nc" not in _CACHE:
        _CACHE["nc"] = build_nc(B)
    nc = _CACHE["nc"]
    in_maps = [
        _prep_inputs(x, q_w, q_b, kv_w, kv_b, proj_w, proj_b, rpb_table,
                     rel_index, core)
        for core in range(NCORES)
    ]
    res = bass_utils.run_bass_kernel_spmd(nc, in_maps, list(range(NCORES)),
                                          trace=False)
    out = np.empty((B, C, N), np.float32)
    for core in range(NCORES):
        o = res.results[core]["out"]  # [B, 3, 128, NI]
        out[:, :, core * NI:(core + 1) * NI] = o.reshape(B, C, NI)
    return out.reshape(B, C, Hh, Ww)
